# revision 1
# baseline (speedup 1.0000x reference)
"""Fused single-head cross-attention on 8 TRN2 NeuronCores (Bass/Tile).

Problem: out = (softmax(norm * (xWq+bq)(yWk+bk)^T + adj) @ (yWv+bv)) Wo + bo
Shapes: x,y [4, 2048, 1024], adj [4, 2048, 2048], all weights [1024, 1024].

Sharding: data-parallel over (batch, seq-half) -> 8 shards. Core c handles
batch b=c//2, query rows h*1024..(h+1)*1024 (h=c%2). K/V projections are
split across the core pair (each computes its own t-half) and exchanged
with pair-wise AllGather collectives, pipelined against later projections.

Layout strategy (zero on-chip transposes; weights pre-tiled on host so
every DMA row is >=2KB contiguous):
  Host pre-transposes activations to feature-major: xT [d1, s], yT [d2, t],
  adjT [t, s]. All attention math runs in "transposed" space:
    KT[d,t]   = matmul(lhsT=Wk, rhs=yT)                  (+bk per-partition)
    V [t,d]   = matmul(lhsT=yT, rhs=Wv)                  (+bv via gpsimd bcast)
    QT[d,s]   = matmul(lhsT=Wq, rhs=xT)                  (+bq per-partition)
    attT[t,s] = matmul(lhsT=KT, rhs=QT)  (+adjT via DVE, exp via ACT)
    numT[d,s] = matmul(lhsT=V,  rhs=exp)   (PSUM, evacuated per t-panel)
    denom[s]  = DVE-accumulated exp + gpsimd partition_all_reduce
    outT[d2,s]= matmul(lhsT=Wo, rhs=numT*recip(denom))   (+bo per-partition)
  softmax max-subtraction is skipped: logits are O(1) by construction.
  All matmul operands are float32r (1 cyc/row vs 4 for fp32; ~1e-4 rel err).
"""
import sys

if "/opt/trn_rl_repo" not in sys.path:
    sys.path.insert(0, "/opt/trn_rl_repo")

import numpy as np

import concourse.bass as bass
import concourse.bass_isa as bass_isa
import concourse.tile as tile
from concourse import bacc, mybir
from concourse.bass_utils import run_bass_kernel_spmd

P = 128
D = 1024
S = 2048
SC = 1024            # per-core query rows; also per-core K/V t-half
DC = D // P          # 8 feature chunks
SB = 512             # matmul moving free dim
NSB = SC // SB       # 2 s blocks
TP = 512             # t panel
NTP = S // TP        # 4 panels
TTP = TP // P        # 4 t-tiles per panel
NORM = 1.0 / 32.0
GROUPS = [[0, 1], [2, 3], [4, 5], [6, 7]]

F32 = mybir.dt.float32
F32R = mybir.dt.float32r
ID = mybir.ActivationFunctionType.Identity
EXP = mybir.ActivationFunctionType.Exp

_CACHE = {}


def _mm(nc, ps, lhsT, rhs, start, stop):
    nc.tensor.matmul(ps, lhsT=lhsT, rhs=rhs, start=start, stop=stop)


def build_nc():
    nc = bacc.Bacc("TRN2", target_bir_lowering=False, debug=False, num_devices=8)

    xT = nc.dram_tensor("xT", [D, SC], F32, kind="ExternalInput")
    yT = nc.dram_tensor("yT", [D, SC], F32, kind="ExternalInput")  # own t-half
    adjT = nc.dram_tensor("adjT", [S, SC], F32, kind="ExternalInput")
    # weights pre-tiled on host: Wx_t[dt][p][c][col] = Wx[c*P+p, dt*P+col]
    Wq = nc.dram_tensor("Wq", [DC, P, DC, P], F32, kind="ExternalInput")
    Wk = nc.dram_tensor("Wk", [DC, P, DC, P], F32, kind="ExternalInput")
    Wo = nc.dram_tensor("Wo", [DC, P, DC, P], F32, kind="ExternalInput")
    # Wv pre-tiled as rhs: Wv_t[db][p][c][col] = Wv[c*P+p, db*SB+col]
    Wv = nc.dram_tensor("Wv", [2, P, DC, SB], F32, kind="ExternalInput")
    bq = nc.dram_tensor("bq", [P, DC], F32, kind="ExternalInput")
    bk = nc.dram_tensor("bk", [P, DC], F32, kind="ExternalInput")
    bv = nc.dram_tensor("bv", [1, D], F32, kind="ExternalInput")
    bo = nc.dram_tensor("bo", [P, DC], F32, kind="ExternalInput")
    outT = nc.dram_tensor("outT", [D, SC], F32, kind="ExternalOutput")

    # local K/V halves + pair-gathered tensors, split by 512-block for
    # finer collective/compute pipelining
    kT_loc = [nc.dram_tensor(f"kT_loc{i}", [D // 2, S // 2], F32R) for i in range(2)]
    v_loc = [nc.dram_tensor(f"v_loc{i}", [SB, D], F32R) for i in range(2)]
    kT_all = [nc.dram_tensor(f"kT_all{i}", [2, D // 2, S // 2], F32R) for i in range(2)]
    v_all = [nc.dram_tensor(f"v_all{i}", [2, SB, D], F32R) for i in range(2)]

    xT_r = xT.rearrange("(c p) s -> p c s", p=P)
    yT_r = yT.rearrange("(c p) t -> p c t", p=P)
    kT_all_r = [t.rearrange("r (c p) t -> r p c t", p=P) for t in kT_all]  # c in 0..3
    v_all_r = [t.rearrange("r (j p) d -> r p j d", p=P) for t in v_all]

    with tile.TileContext(nc) as tc:
        with (
            nc.allow_low_precision(reason="float32r is bit-identical to fp32"),
            tc.tile_pool(name="res", bufs=1) as res,
        ):
            # ---- resident tiles --------------------------------------
            QT_sb = res.tile([P, DC, SC], F32R, name="QT_sb")
            num_sb = res.tile([P, DC, SC], F32, name="num_sb")
            recip_sb = res.tile([1, NSB, SB], F32, name="recip_sb")
            bv_bc = res.tile([P, D], F32, name="bv_bc")
            bq_sb = res.tile([P, DC], F32, name="bq_sb")
            bk_sb = res.tile([P, DC], F32, name="bk_sb")
            bo_sb = res.tile([P, DC], F32, name="bo_sb")
            bv_sb = res.tile([1, D], F32, name="bv_sb")
            nc.sync.dma_start(out=bk_sb[:], in_=bk[:])
            nc.sync.dma_start(out=bv_sb[:], in_=bv[:])
            nc.sync.dma_start(out=bq_sb[:], in_=bq[:])
            nc.sync.dma_start(out=bo_sb[:], in_=bo[:])
            nc.gpsimd.partition_broadcast(bv_bc[:], bv_sb[0:1, :], channels=P)

            with (
                tc.tile_pool(name="qkv_in", bufs=1) as qkvp,
                tc.tile_pool(name="w_pool", bufs=4) as wp,
                tc.tile_pool(name="wv_pool", bufs=1) as wvp,
                tc.tile_pool(name="kv_out", bufs=3) as kvo,
                tc.tile_pool(name="qkv_ps", bufs=3, space="PSUM") as qps,
            ):
                yT_sb = qkvp.tile([P, DC, SC], F32R, name="yT_sb")
                xT_sb = qkvp.tile([P, DC, SC], F32R, name="xT_sb")
                wv_t = [wvp.tile([P, DC, SB], F32R, name=f"wv{i}") for i in range(2)]
                for c in range(DC):
                    for hh in range(2):
                        hsl = slice(hh * SB, (hh + 1) * SB)
                        nc.sync.dma_start(
                            out=yT_sb[:, c, hsl], in_=yT_r[:, c, hsl].bitcast(F32R)
                        )

                def emit_late_inputs():
                    for db in range(2):
                        for ch in range(4):
                            csl = slice(ch * 2, (ch + 1) * 2)
                            nc.sync.dma_start(
                                out=wv_t[db][:, csl, :],
                                in_=Wv[db, :, csl, :].bitcast(F32R),
                            )
                    for c in range(DC):
                        nc.sync.dma_start(
                            out=xT_sb[:, c, :], in_=xT_r[:, c, :].bitcast(F32R)
                        )

                def emit_k(dh):
                    for dt in range(dh * 4, dh * 4 + 4):
                        wk = wp.tile([P, DC, P], F32R, name="wk_t", tag="w")
                        for ch in range(2):
                            csl = slice(ch * 4, (ch + 1) * 4)
                            nc.sync.dma_start(
                                out=wk[:, csl, :], in_=Wk[dt, :, csl, :].bitcast(F32R)
                            )
                        for tb in range(NSB):
                            ps = qps.tile([P, SB], F32, name="k_ps", tag="qkvps")
                            for c in range(DC):
                                _mm(
                                    nc, ps[:],
                                    wk[:, c, :],
                                    yT_sb[:, c, tb * SB : (tb + 1) * SB],
                                    c == 0, c == DC - 1,
                                )
                            kt = kvo.tile([P, SB], F32R, name="kt")
                            nc.scalar.activation(
                                out=kt[:], in_=ps[:], func=ID,
                                bias=bk_sb[:, dt : dt + 1],
                            )
                            nc.sync.dma_start(
                                out=kT_loc[dh][(dt - dh * 4) * P : (dt - dh * 4 + 1) * P,
                                               tb * SB : (tb + 1) * SB],
                                in_=kt[:],
                            )
                    nc.gpsimd.collective_compute(
                        "AllGather", mybir.AluOpType.bypass,
                        replica_groups=GROUPS,
                        ins=[kT_loc[dh][:]], outs=[kT_all[dh][:]],
                    )

                def emit_v(tb):
                    for tl in range(SB // P):
                        tt = tb * (SB // P) + tl
                        for db in range(2):
                            ps = qps.tile([P, SB], F32, name="v_ps", tag="qkvps")
                            for c in range(DC):
                                _mm(
                                    nc, ps[:],
                                    yT_sb[:, c, tt * P : (tt + 1) * P],
                                    wv_t[db][:, c, :],
                                    c == 0, c == DC - 1,
                                )
                            vt = kvo.tile([P, SB], F32R, name="vt")
                            nc.vector.tensor_add(
                                vt[:], ps[:], bv_bc[:, db * SB : (db + 1) * SB]
                            )
                            nc.sync.dma_start(
                                out=v_loc[tb][tl * P : (tl + 1) * P,
                                              db * SB : (db + 1) * SB],
                                in_=vt[:],
                            )
                    nc.gpsimd.collective_compute(
                        "AllGather", mybir.AluOpType.bypass,
                        replica_groups=GROUPS,
                        ins=[v_loc[tb][:]], outs=[v_all[tb][:]],
                    )

                emit_k(0)
                emit_late_inputs()
                emit_k(1)
                emit_v(0)
                emit_v(1)

                # ---- phase Q: QT = Wq^T x^T + bq ---------------------
                for dt in range(DC):
                    wq = wp.tile([P, DC, P], F32R, name="wq_t", tag="w")
                    for ch in range(2):
                        csl = slice(ch * 4, (ch + 1) * 4)
                        nc.sync.dma_start(
                            out=wq[:, csl, :], in_=Wq[dt, :, csl, :].bitcast(F32R)
                        )
                    for sb in range(NSB):
                        ps = qps.tile([P, SB], F32, name="q_ps", tag="qkvps")
                        for c in range(DC):
                            _mm(
                                nc, ps[:],
                                wq[:, c, :],
                                xT_sb[:, c, sb * SB : (sb + 1) * SB],
                                c == 0, c == DC - 1,
                            )
                        nc.scalar.activation(
                            out=QT_sb[:, dt, sb * SB : (sb + 1) * SB],
                            in_=ps[:], func=ID, bias=bq_sb[:, dt : dt + 1],
                        )

            # ---- phase A: attention, t-panel outer -------------------
            with tc.tile_pool(name="late_res", bufs=1) as lres:
              denacc = lres.tile([P, NSB, SB], F32, name="denacc")
              dsum = lres.tile([P, SB], F32, name="dsum")
              rb = lres.tile([P, NSB, SB], F32, name="rb")
              scaled = lres.tile([P, NSB, DC, SB], F32R, name="scaled")
              with (
                tc.tile_pool(name="kp_pool", bufs=2) as kpp,
                tc.tile_pool(name="vp_pool", bufs=2) as vpp,
                tc.tile_pool(name="exp_pool", bufs=2) as expp,
                tc.tile_pool(name="adj_pool", bufs=2) as adjp,
                tc.tile_pool(name="tmp_pool", bufs=2) as tmpp,
                tc.tile_pool(name="aps", bufs=3, space="PSUM") as aps,
                tc.tile_pool(name="nps", bufs=5, space="PSUM") as npsp,
              ):
                for panel in range(NTP):
                    r, lb = panel // 2, panel % 2
                    kp = kpp.tile([P, DC, TP], F32R, name="kp")
                    for c in range(DC):
                        nc.sync.dma_start(
                            out=kp[:, c, :],
                            in_=kT_all_r[c // 4][r, :, c % 4,
                                                 lb * TP : (lb + 1) * TP],
                        )
                    vp = vpp.tile([P, TTP, D], F32R, name="vp")
                    for j in range(TTP):
                        nc.sync.dma_start(
                            out=vp[:, j, :], in_=v_all_r[lb][r, :, j, :]
                        )
                    for sb in range(NSB):
                        ssl = slice(sb * SB, (sb + 1) * SB)
                        ex = expp.tile([P, TTP, SB], F32R, name="ex")
                        for tt in range(TTP):
                            tg = panel * TTP + tt
                            att = aps.tile([P, SB], F32, name="att")
                            for c in range(DC):
                                _mm(
                                    nc, att[:],
                                    kp[:, c, tt * P : (tt + 1) * P],
                                    QT_sb[:, c, ssl],
                                    c == 0, c == DC - 1,
                                )
                            at = adjp.tile([P, SB], F32, name="at")
                            nc.sync.dma_start(
                                out=at[:], in_=adjT[tg * P : (tg + 1) * P, ssl]
                            )
                            tm = tmpp.tile([P, SB], F32, name="tm")
                            nc.vector.tensor_add(tm[:], att[:], at[:])
                            nc.scalar.activation(
                                out=ex[:, tt, :], in_=tm[:], func=EXP
                            )
                            if panel == 0 and tt == 0:
                                nc.vector.tensor_copy(denacc[:, sb, :], ex[:, tt, :])
                            else:
                                nc.vector.tensor_add(
                                    denacc[:, sb, :], denacc[:, sb, :], ex[:, tt, :]
                                )
                        # numT partial for this panel, d split in halves
                        for dh in range(2):
                            nt = [
                                npsp.tile([P, SB], F32, name="np")
                                for _ in range(DC // 2)
                            ]
                            for tt in range(TTP):
                                for d4 in range(DC // 2):
                                    _mm(
                                        nc, nt[d4][:],
                                        vp[:, tt,
                                           (dh * 4 + d4) * P : (dh * 4 + d4 + 1) * P],
                                        ex[:, tt, :],
                                        tt == 0, tt == TTP - 1,
                                    )
                            for d4 in range(DC // 2):
                                dst = num_sb[:, dh * 4 + d4, ssl]
                                if panel == 0:
                                    nc.vector.tensor_copy(dst, nt[d4][:])
                                else:
                                    nc.vector.tensor_add(dst, dst, nt[d4][:])
                        if panel == NTP - 1:
                            # finalize softmax scale for this s-block while
                            # the other s-block still computes
                            nc.gpsimd.partition_all_reduce(
                                dsum[:], denacc[:, sb, :],
                                channels=P, reduce_op=bass_isa.ReduceOp.add,
                            )
                            nc.vector.reciprocal(recip_sb[0:1, sb, :], dsum[0:1, :])
                            nc.gpsimd.partition_broadcast(
                                rb[:, sb, :], recip_sb[0:1, sb, :], channels=P
                            )
                            for c in range(DC):
                                nc.vector.tensor_mul(
                                    scaled[:, sb, c, :],
                                    num_sb[:, c, ssl],
                                    rb[:, sb, :],
                                )

              # ---- phase O: out^T = Wo^T (numT*recip) + bo ---------
              with (
                  tc.tile_pool(name="wo_pool", bufs=3) as wop,
                  tc.tile_pool(name="o_out", bufs=3) as oout,
                  tc.tile_pool(name="ops", bufs=3, space="PSUM") as ops,
              ):
                  for dt in range(DC):
                      wo_t = wop.tile([P, DC, P], F32R, name="wo_t")
                      nc.sync.dma_start(out=wo_t[:], in_=Wo[dt].bitcast(F32R))
                      for sb in range(NSB):
                          po = ops.tile([P, SB], F32, name="po")
                          for c in range(DC):
                              _mm(
                                  nc, po[:],
                                  wo_t[:, c, :],
                                  scaled[:, sb, c, :],
                                  c == 0, c == DC - 1,
                              )
                          ot = oout.tile([P, SB], F32, name="ot")
                          nc.scalar.activation(
                              out=ot[:], in_=po[:], func=ID,
                              bias=bo_sb[:, dt : dt + 1],
                          )
                          nc.sync.dma_start(
                              out=outT[dt * P : (dt + 1) * P,
                                       sb * SB : (sb + 1) * SB],
                              in_=ot[:],
                          )
    nc.compile()
    return nc


def _get_nc():
    if "nc" not in _CACHE:
        _CACHE["nc"] = build_nc()
    return _CACHE["nc"]


def _tile_lhs(W):
    # [dt][p][c][col] = W[c*P+p, dt*P+col]
    return np.ascontiguousarray(
        W.reshape(DC, P, DC, P).transpose(2, 1, 0, 3)
    )


def kernel(x, y, adj, Wq, bq, Wk, bk, Wv, bv, Wo, bo, _trace=False):
    x = np.asarray(x, dtype=np.float32)
    y = np.asarray(y, dtype=np.float32)
    adj = np.asarray(adj, dtype=np.float32)
    Wq_h = _tile_lhs(np.asarray(Wq, np.float32) * NORM)
    Wk_h = _tile_lhs(np.asarray(Wk, np.float32))
    Wo_h = _tile_lhs(np.asarray(Wo, np.float32))
    # Wv as rhs tiles: [db][p][c][col] = Wv[c*P+p, db*SB+col]
    Wv_h = np.ascontiguousarray(
        np.asarray(Wv, np.float32).reshape(DC, P, 2, SB).transpose(2, 1, 0, 3)
    )
    bq_s = np.asarray(bq, np.float32) * NORM
    bq_h = np.ascontiguousarray(bq_s.reshape(DC, P).T)
    bk_h = np.ascontiguousarray(np.asarray(bk, np.float32).reshape(DC, P).T)
    bo_h = np.ascontiguousarray(np.asarray(bo, np.float32).reshape(DC, P).T)
    bv_h = np.ascontiguousarray(np.asarray(bv, np.float32).reshape(1, D))

    in_maps = []
    for c in range(8):
        b, h = c // 2, c % 2
        ssl = slice(h * SC, (h + 1) * SC)
        in_maps.append(
            {
                "xT": np.ascontiguousarray(x[b, ssl, :].T),
                "yT": np.ascontiguousarray(y[b, ssl, :].T),
                "adjT": np.ascontiguousarray(adj[b, ssl, :].T),
                "Wq": Wq_h, "Wk": Wk_h, "Wv": Wv_h, "Wo": Wo_h,
                "bq": bq_h, "bk": bk_h, "bv": bv_h, "bo": bo_h,
            }
        )

    nc = _get_nc()
    res = run_bass_kernel_spmd(nc, in_maps, list(range(8)), trace=_trace)
    if _trace:
        _CACHE["last_exec_time_ns"] = res.exec_time_ns
        _CACHE["last_trace"] = (
            res.instructions_and_trace[1] if res.instructions_and_trace else None
        )

    out = np.empty((4, S, D), np.float32)
    for c in range(8):
        b, h = c // 2, c % 2
        out[b, h * SC : (h + 1) * SC, :] = res.results[c]["outT"].T
    return out



# revision 2
# speedup vs baseline: 1.4177x; 1.4177x over previous
"""Fused single-head cross-attention on 8 TRN2 NeuronCores (Bass/Tile).

Problem: out = (softmax(norm * (xWq+bq)(yWk+bk)^T + adj) @ (yWv+bv)) Wo + bo
Shapes: x,y [4, 2048, 1024], adj [4, 2048, 2048], all weights [1024, 1024].

Sharding: data-parallel over (batch, seq-half) -> 8 shards. Core c handles
batch b=c//2, query rows h*1024..(h+1)*1024 (h=c%2). K/V projections are
split across the core pair (each computes its own t-half) and exchanged
with pair-wise AllGather collectives, pipelined against later projections.

Layout strategy (zero on-chip transposes; weights pre-tiled on host so
every DMA row is >=1KB contiguous):
  Host pre-transposes activations to feature-major: xT [d1, s], yT [d2, t],
  adjT [t, s]. All attention math runs in "transposed" space:
    KT[d,t]   = matmul(lhsT=Wk, rhs=yT)                  (+bk per-partition)
    V [t,d]   = matmul(lhsT=yT, rhs=Wv)                  (+bv via gpsimd bcast)
    QT[d,s]   = matmul(lhsT=Wq, rhs=xT)                  (+bq per-partition)
    attT[t,s] = matmul(lhsT=KT, rhs=QT)  (+adjT via DVE, exp via ACT)
    numT[d,s] = matmul(lhsT=V,  rhs=exp)   (PSUM, evacuated per t-panel)
    denom[s]  = DVE-accumulated exp + gpsimd partition_all_reduce
    outT[d2,s]= matmul(lhsT=Wo, rhs=numT*recip(denom))   (+bo per-partition)
  softmax max-subtraction is skipped: logits are O(1) by construction.
  All matmul operands are bf16 (2x tensor throughput vs fp32, halved DMA
  and collective bytes); PSUM accumulation stays fp32. Measured rel err
  ~5e-3 vs the 2e-2 gate.
"""
import sys

if "/opt/trn_rl_repo" not in sys.path:
    sys.path.insert(0, "/opt/trn_rl_repo")

import ml_dtypes
import numpy as np

import concourse.bass as bass
import concourse.bass_isa as bass_isa
import concourse.tile as tile
from concourse import bacc, mybir
from concourse.bass_utils import run_bass_kernel_spmd

P = 128
D = 1024
S = 2048
SC = 1024            # per-core query rows; also per-core K/V t-half
DC = D // P          # 8 feature chunks
SB = 512             # matmul moving free dim
NSB = SC // SB       # 2 s blocks
TP = 512             # t panel
NTP = S // TP        # 4 panels
TTP = TP // P        # 4 t-tiles per panel
NORM = 1.0 / 32.0
GROUPS = [[0, 1], [2, 3], [4, 5], [6, 7]]

F32 = mybir.dt.float32
BF16 = mybir.dt.bfloat16
BF16NP = ml_dtypes.bfloat16
ID = mybir.ActivationFunctionType.Identity
EXP = mybir.ActivationFunctionType.Exp

_CACHE = {}


def _mm(nc, ps, lhsT, rhs, start, stop):
    nc.tensor.matmul(ps, lhsT=lhsT, rhs=rhs, start=start, stop=stop)


def build_nc():
    nc = bacc.Bacc("TRN2", target_bir_lowering=False, debug=False, num_devices=8)

    xT = nc.dram_tensor("xT", [D, SC], BF16, kind="ExternalInput")
    yT = nc.dram_tensor("yT", [D, SC], BF16, kind="ExternalInput")  # own t-half
    adjT = nc.dram_tensor("adjT", [S, SC], BF16, kind="ExternalInput")
    # weights pre-tiled on host: Wx_t[dt][p][c][col] = Wx[c*P+p, dt*P+col]
    Wq = nc.dram_tensor("Wq", [DC, P, DC, P], BF16, kind="ExternalInput")
    Wk = nc.dram_tensor("Wk", [DC, P, DC, P], BF16, kind="ExternalInput")
    Wo = nc.dram_tensor("Wo", [DC, P, DC, P], BF16, kind="ExternalInput")
    # Wv pre-tiled as rhs: Wv_t[db][p][c][col] = Wv[c*P+p, db*SB+col]
    Wv = nc.dram_tensor("Wv", [2, P, DC, SB], BF16, kind="ExternalInput")
    bq = nc.dram_tensor("bq", [P, DC], F32, kind="ExternalInput")
    bk = nc.dram_tensor("bk", [P, DC], F32, kind="ExternalInput")
    bv = nc.dram_tensor("bv", [1, D], F32, kind="ExternalInput")
    bo = nc.dram_tensor("bo", [P, DC], F32, kind="ExternalInput")
    outT = nc.dram_tensor("outT", [D, SC], F32, kind="ExternalOutput")

    # local K/V halves + pair-gathered tensors, split by 512-block for
    # finer collective/compute pipelining
    kT_loc = [nc.dram_tensor(f"kT_loc{i}", [D // 2, S // 2], BF16) for i in range(2)]
    v_loc = [nc.dram_tensor(f"v_loc{i}", [SB, D], BF16) for i in range(2)]
    kT_all = [nc.dram_tensor(f"kT_all{i}", [2, D // 2, S // 2], BF16) for i in range(2)]
    v_all = [nc.dram_tensor(f"v_all{i}", [2, SB, D], BF16) for i in range(2)]

    xT_r = xT.rearrange("(c p) s -> p c s", p=P)
    yT_r = yT.rearrange("(c p) t -> p c t", p=P)
    kT_all_r = [t.rearrange("r (c p) t -> r p c t", p=P) for t in kT_all]  # c in 0..3
    v_all_r = [t.rearrange("r (j p) d -> r p j d", p=P) for t in v_all]

    with tile.TileContext(nc) as tc:
        with (
            nc.allow_low_precision(reason="bf16 matmuls, fp32 accumulate"),
            tc.tile_pool(name="res", bufs=1) as res,
        ):
            # ---- resident tiles --------------------------------------
            QT_sb = res.tile([P, DC, SC], BF16, name="QT_sb")
            num_sb = res.tile([P, DC, SC], F32, name="num_sb")
            recip_sb = res.tile([1, NSB, SB], F32, name="recip_sb")
            bv_bc = res.tile([P, D], F32, name="bv_bc")
            bq_sb = res.tile([P, DC], F32, name="bq_sb")
            bk_sb = res.tile([P, DC], F32, name="bk_sb")
            bo_sb = res.tile([P, DC], F32, name="bo_sb")
            bv_sb = res.tile([1, D], F32, name="bv_sb")
            nc.sync.dma_start(out=bk_sb[:], in_=bk[:])
            nc.sync.dma_start(out=bv_sb[:], in_=bv[:])
            nc.sync.dma_start(out=bq_sb[:], in_=bq[:])
            nc.sync.dma_start(out=bo_sb[:], in_=bo[:])
            nc.gpsimd.partition_broadcast(bv_bc[:], bv_sb[0:1, :], channels=P)

            with (
                tc.tile_pool(name="qkv_in", bufs=1) as qkvp,
                tc.tile_pool(name="w_pool", bufs=4) as wp,
                tc.tile_pool(name="wv_pool", bufs=1) as wvp,
                tc.tile_pool(name="kv_out", bufs=3) as kvo,
                tc.tile_pool(name="qkv_ps", bufs=3, space="PSUM") as qps,
            ):
                yT_sb = qkvp.tile([P, DC, SC], BF16, name="yT_sb")
                xT_sb = qkvp.tile([P, DC, SC], BF16, name="xT_sb")
                wv_t = [wvp.tile([P, DC, SB], BF16, name=f"wv{i}") for i in range(2)]
                for c in range(DC):
                    for hh in range(2):
                        hsl = slice(hh * SB, (hh + 1) * SB)
                        nc.sync.dma_start(
                            out=yT_sb[:, c, hsl], in_=yT_r[:, c, hsl]
                        )

                def emit_late_inputs():
                    for db in range(2):
                        for ch in range(4):
                            csl = slice(ch * 2, (ch + 1) * 2)
                            nc.sync.dma_start(
                                out=wv_t[db][:, csl, :],
                                in_=Wv[db, :, csl, :],
                            )
                    for c in range(DC):
                        nc.sync.dma_start(
                            out=xT_sb[:, c, :], in_=xT_r[:, c, :]
                        )

                def emit_k(dh):
                    for dt in range(dh * 4, dh * 4 + 4):
                        wk = wp.tile([P, DC, P], BF16, name="wk_t", tag="w")
                        for ch in range(2):
                            csl = slice(ch * 4, (ch + 1) * 4)
                            nc.sync.dma_start(
                                out=wk[:, csl, :], in_=Wk[dt, :, csl, :]
                            )
                        for tb in range(NSB):
                            ps = qps.tile([P, SB], F32, name="k_ps", tag="qkvps")
                            for c in range(DC):
                                _mm(
                                    nc, ps[:],
                                    wk[:, c, :],
                                    yT_sb[:, c, tb * SB : (tb + 1) * SB],
                                    c == 0, c == DC - 1,
                                )
                            kt = kvo.tile([P, SB], BF16, name="kt")
                            nc.scalar.activation(
                                out=kt[:], in_=ps[:], func=ID,
                                bias=bk_sb[:, dt : dt + 1],
                            )
                            nc.sync.dma_start(
                                out=kT_loc[dh][(dt - dh * 4) * P : (dt - dh * 4 + 1) * P,
                                               tb * SB : (tb + 1) * SB],
                                in_=kt[:],
                            )
                    nc.gpsimd.collective_compute(
                        "AllGather", mybir.AluOpType.bypass,
                        replica_groups=GROUPS,
                        ins=[kT_loc[dh][:]], outs=[kT_all[dh][:]],
                    )

                def emit_v(tb):
                    for tl in range(SB // P):
                        tt = tb * (SB // P) + tl
                        for db in range(2):
                            ps = qps.tile([P, SB], F32, name="v_ps", tag="qkvps")
                            for c in range(DC):
                                _mm(
                                    nc, ps[:],
                                    yT_sb[:, c, tt * P : (tt + 1) * P],
                                    wv_t[db][:, c, :],
                                    c == 0, c == DC - 1,
                                )
                            vt = kvo.tile([P, SB], BF16, name="vt")
                            nc.vector.tensor_add(
                                vt[:], ps[:], bv_bc[:, db * SB : (db + 1) * SB]
                            )
                            nc.sync.dma_start(
                                out=v_loc[tb][tl * P : (tl + 1) * P,
                                              db * SB : (db + 1) * SB],
                                in_=vt[:],
                            )
                    nc.gpsimd.collective_compute(
                        "AllGather", mybir.AluOpType.bypass,
                        replica_groups=GROUPS,
                        ins=[v_loc[tb][:]], outs=[v_all[tb][:]],
                    )

                emit_k(0)
                emit_late_inputs()
                emit_k(1)
                emit_v(0)
                emit_v(1)

                # ---- phase Q: QT = Wq^T x^T + bq ---------------------
                for dt in range(DC):
                    wq = wp.tile([P, DC, P], BF16, name="wq_t", tag="w")
                    for ch in range(2):
                        csl = slice(ch * 4, (ch + 1) * 4)
                        nc.sync.dma_start(
                            out=wq[:, csl, :], in_=Wq[dt, :, csl, :]
                        )
                    for sb in range(NSB):
                        ps = qps.tile([P, SB], F32, name="q_ps", tag="qkvps")
                        for c in range(DC):
                            _mm(
                                nc, ps[:],
                                wq[:, c, :],
                                xT_sb[:, c, sb * SB : (sb + 1) * SB],
                                c == 0, c == DC - 1,
                            )
                        nc.scalar.activation(
                            out=QT_sb[:, dt, sb * SB : (sb + 1) * SB],
                            in_=ps[:], func=ID, bias=bq_sb[:, dt : dt + 1],
                        )

            # ---- phase A: attention, t-panel outer -------------------
            # order (r, lb): consume v_all[0] panels first so the second
            # V AllGather has Q-proj + 2 panels of slack to land
            PANELS = [(0, 0), (1, 0), (0, 1), (1, 1)]
            with tc.tile_pool(name="late_res", bufs=1) as lres:
              denacc = lres.tile([P, NSB, SB], F32, name="denacc")
              dsum = lres.tile([P, SB], F32, name="dsum")
              rb = lres.tile([P, NSB, SB], F32, name="rb")
              scaled = lres.tile([P, NSB, DC, SB], BF16, name="scaled")
              with (
                tc.tile_pool(name="kp_pool", bufs=2) as kpp,
                tc.tile_pool(name="vp_pool", bufs=2) as vpp,
                tc.tile_pool(name="exp_pool", bufs=2) as expp,
                tc.tile_pool(name="adj_pool", bufs=2) as adjp,
                tc.tile_pool(name="tmp_pool", bufs=2) as tmpp,
                tc.tile_pool(name="aps", bufs=3, space="PSUM") as aps,
                tc.tile_pool(name="nps", bufs=5, space="PSUM") as npsp,
              ):
                for pi, (r, lb) in enumerate(PANELS):
                    kp = kpp.tile([P, DC, TP], BF16, name="kp")
                    for c in range(DC):
                        nc.sync.dma_start(
                            out=kp[:, c, :],
                            in_=kT_all_r[c // 4][r, :, c % 4,
                                                 lb * TP : (lb + 1) * TP],
                        )
                    vp = vpp.tile([P, TTP, D], BF16, name="vp")
                    for j in range(TTP):
                        nc.sync.dma_start(
                            out=vp[:, j, :], in_=v_all_r[lb][r, :, j, :]
                        )
                    for sb in range(NSB):
                        ssl = slice(sb * SB, (sb + 1) * SB)
                        ex = expp.tile([P, TTP, SB], BF16, name="ex")
                        for tt in range(TTP):
                            tg = r * 8 + lb * 4 + tt
                            att = aps.tile([P, SB], F32, name="att")
                            for c in range(DC):
                                _mm(
                                    nc, att[:],
                                    kp[:, c, tt * P : (tt + 1) * P],
                                    QT_sb[:, c, ssl],
                                    c == 0, c == DC - 1,
                                )
                            at = adjp.tile([P, SB], BF16, name="at")
                            nc.sync.dma_start(
                                out=at[:], in_=adjT[tg * P : (tg + 1) * P, ssl]
                            )
                            tm = tmpp.tile([P, SB], F32, name="tm")
                            nc.vector.tensor_add(tm[:], att[:], at[:])
                            nc.scalar.activation(
                                out=ex[:, tt, :], in_=tm[:], func=EXP
                            )
                            if pi == 0 and tt == 0:
                                nc.vector.tensor_copy(denacc[:, sb, :], ex[:, tt, :])
                            else:
                                nc.vector.tensor_add(
                                    denacc[:, sb, :], denacc[:, sb, :], ex[:, tt, :]
                                )
                        # numT partial for this panel, d split in halves
                        for dh in range(2):
                            nt = [
                                npsp.tile([P, SB], F32, name="np")
                                for _ in range(DC // 2)
                            ]
                            for tt in range(TTP):
                                for d4 in range(DC // 2):
                                    _mm(
                                        nc, nt[d4][:],
                                        vp[:, tt,
                                           (dh * 4 + d4) * P : (dh * 4 + d4 + 1) * P],
                                        ex[:, tt, :],
                                        tt == 0, tt == TTP - 1,
                                    )
                            for d4 in range(DC // 2):
                                dst = num_sb[:, dh * 4 + d4, ssl]
                                if pi == 0:
                                    nc.vector.tensor_copy(dst, nt[d4][:])
                                else:
                                    nc.vector.tensor_add(dst, dst, nt[d4][:])
                        if pi == NTP - 1:
                            # finalize softmax scale for this s-block while
                            # the other s-block still computes
                            nc.gpsimd.partition_all_reduce(
                                dsum[:], denacc[:, sb, :],
                                channels=P, reduce_op=bass_isa.ReduceOp.add,
                            )
                            nc.vector.reciprocal(recip_sb[0:1, sb, :], dsum[0:1, :])
                            nc.gpsimd.partition_broadcast(
                                rb[:, sb, :], recip_sb[0:1, sb, :], channels=P
                            )
                            for c in range(DC):
                                nc.vector.tensor_mul(
                                    scaled[:, sb, c, :],
                                    num_sb[:, c, ssl],
                                    rb[:, sb, :],
                                )

              # ---- phase O: out^T = Wo^T (numT*recip) + bo ---------
              with (
                  tc.tile_pool(name="wo_pool", bufs=3) as wop,
                  tc.tile_pool(name="o_out", bufs=3) as oout,
                  tc.tile_pool(name="ops", bufs=3, space="PSUM") as ops,
              ):
                  for dt in range(DC):
                      wo_t = wop.tile([P, DC, P], BF16, name="wo_t")
                      nc.sync.dma_start(out=wo_t[:], in_=Wo[dt])
                      for sb in range(NSB):
                          po = ops.tile([P, SB], F32, name="po")
                          for c in range(DC):
                              _mm(
                                  nc, po[:],
                                  wo_t[:, c, :],
                                  scaled[:, sb, c, :],
                                  c == 0, c == DC - 1,
                              )
                          ot = oout.tile([P, SB], F32, name="ot")
                          nc.scalar.activation(
                              out=ot[:], in_=po[:], func=ID,
                              bias=bo_sb[:, dt : dt + 1],
                          )
                          nc.sync.dma_start(
                              out=outT[dt * P : (dt + 1) * P,
                                       sb * SB : (sb + 1) * SB],
                              in_=ot[:],
                          )
    nc.compile()
    return nc


def _get_nc():
    if "nc" not in _CACHE:
        _CACHE["nc"] = build_nc()
    return _CACHE["nc"]


def _tile_lhs(W):
    # [dt][p][c][col] = W[c*P+p, dt*P+col]
    return np.ascontiguousarray(
        W.reshape(DC, P, DC, P).transpose(2, 1, 0, 3).astype(BF16NP)
    )


def kernel(x, y, adj, Wq, bq, Wk, bk, Wv, bv, Wo, bo, _trace=False):
    x = np.asarray(x, dtype=np.float32)
    y = np.asarray(y, dtype=np.float32)
    adj = np.asarray(adj, dtype=np.float32)
    Wq_h = _tile_lhs(np.asarray(Wq, np.float32) * NORM)
    Wk_h = _tile_lhs(np.asarray(Wk, np.float32))
    Wo_h = _tile_lhs(np.asarray(Wo, np.float32))
    # Wv as rhs tiles: [db][p][c][col] = Wv[c*P+p, db*SB+col]
    Wv_h = np.ascontiguousarray(
        np.asarray(Wv, np.float32).reshape(DC, P, 2, SB).transpose(2, 1, 0, 3)
        .astype(BF16NP)
    )
    bq_s = np.asarray(bq, np.float32) * NORM
    bq_h = np.ascontiguousarray(bq_s.reshape(DC, P).T)
    bk_h = np.ascontiguousarray(np.asarray(bk, np.float32).reshape(DC, P).T)
    bo_h = np.ascontiguousarray(np.asarray(bo, np.float32).reshape(DC, P).T)
    bv_h = np.ascontiguousarray(np.asarray(bv, np.float32).reshape(1, D))

    in_maps = []
    for c in range(8):
        b, h = c // 2, c % 2
        ssl = slice(h * SC, (h + 1) * SC)
        in_maps.append(
            {
                "xT": np.ascontiguousarray(x[b, ssl, :].T.astype(BF16NP)),
                "yT": np.ascontiguousarray(y[b, ssl, :].T.astype(BF16NP)),
                "adjT": np.ascontiguousarray(adj[b, ssl, :].T.astype(BF16NP)),
                "Wq": Wq_h, "Wk": Wk_h, "Wv": Wv_h, "Wo": Wo_h,
                "bq": bq_h, "bk": bk_h, "bv": bv_h, "bo": bo_h,
            }
        )

    nc = _get_nc()
    res = run_bass_kernel_spmd(nc, in_maps, list(range(8)), trace=_trace)
    if _trace:
        _CACHE["last_exec_time_ns"] = res.exec_time_ns
        _CACHE["last_trace"] = (
            res.instructions_and_trace[1] if res.instructions_and_trace else None
        )

    out = np.empty((4, S, D), np.float32)
    for c in range(8):
        b, h = c // 2, c % 2
        out[b, h * SC : (h + 1) * SC, :] = res.results[c]["outT"].T
    return out


# revision 3
# speedup vs baseline: 1.5074x; 1.0633x over previous
"""Fused single-head cross-attention on 8 TRN2 NeuronCores (Bass/Tile).

Problem: out = (softmax(norm * (xWq+bq)(yWk+bk)^T + adj) @ (yWv+bv)) Wo + bo
Shapes: x,y [4, 2048, 1024], adj [4, 2048, 2048], all weights [1024, 1024].

Sharding: data-parallel over (batch, seq-half) -> 8 shards. Core c handles
batch b=c//2, query rows h*1024..(h+1)*1024 (h=c%2). K/V projections are
split across the core pair (each computes its own t-half) and exchanged
with pair-wise AllGather collectives, pipelined against later projections.

Layout strategy (zero on-chip transposes; weights pre-tiled on host):
  Host pre-transposes activations to feature-major: xT [d1, s], yT [d2, t],
  adjT [t, s]. All attention math runs in "transposed" space:
    KT[d,t]   = matmul(lhsT=Wk, rhs=yT)                  (+bk per-partition)
    V [t,d]   = matmul(lhsT=yT, rhs=Wv)                  (+bv via gpsimd bcast)
    QT[d,s]   = matmul(lhsT=Wq, rhs=xT)                  (+bq per-partition)
    attT[t,s] = matmul(lhsT=KT, rhs=QT)  (+adjT via DVE, exp via ACT)
    numT[d,s] = matmul(lhsT=V,  rhs=exp)   (PSUM, evacuated per t-panel)
    denom[s]  = DVE-accumulated exp + gpsimd partition_all_reduce
    outT[d2,s]= matmul(lhsT=Wo, rhs=numT*recip(denom))   (+bo per-partition)
  softmax max-subtraction is skipped: logits are O(1) by construction.
  All matmul operands are bf16 (2x tensor throughput vs fp32, halved DMA
  and collective bytes); PSUM accumulation stays fp32.

DMA discipline (the sync engine executes one ~0.6us trigger per DMA
instruction, in order): all input loads are batched into ~15 big triggers
issued up front; evacuation stores ride the scalar HWDGE queue right
behind the ACTs that produce them; per-panel loads are 3-4 triggers each.
The softmax finalize chain is emitted after the attention loop so it
cannot head-of-line-block the DVE queue mid-phase.
"""
import sys

if "/opt/trn_rl_repo" not in sys.path:
    sys.path.insert(0, "/opt/trn_rl_repo")

import ml_dtypes
import numpy as np

import concourse.bass as bass
import concourse.bass_isa as bass_isa
import concourse.tile as tile
from concourse import bacc, mybir
from concourse.bass_utils import run_bass_kernel_spmd

P = 128
D = 1024
S = 2048
SC = 1024            # per-core query rows; also per-core K/V t-half
DC = D // P          # 8 feature chunks
SB = 512             # matmul moving free dim
NSB = SC // SB       # 2 s blocks
TP = 512             # t panel
NTP = S // TP        # 4 panels
TTP = TP // P        # 4 t-tiles per panel
NORM = 1.0 / 32.0
GROUPS = [[0, 1], [2, 3], [4, 5], [6, 7]]

F32 = mybir.dt.float32
BF16 = mybir.dt.bfloat16
BF16NP = ml_dtypes.bfloat16
ID = mybir.ActivationFunctionType.Identity
EXP = mybir.ActivationFunctionType.Exp

_CACHE = {}


def _mm(nc, ps, lhsT, rhs, start, stop):
    nc.tensor.matmul(ps, lhsT=lhsT, rhs=rhs, start=start, stop=stop)


def build_nc():
    nc = bacc.Bacc("TRN2", target_bir_lowering=False, debug=False, num_devices=8)

    xT = nc.dram_tensor("xT", [D, SC], BF16, kind="ExternalInput")
    yT = nc.dram_tensor("yT", [D, SC], BF16, kind="ExternalInput")  # own t-half
    adjT = nc.dram_tensor("adjT", [S, SC], BF16, kind="ExternalInput")
    # weights pre-tiled on host: Wx_t[dt][p][c][col] = Wx[c*P+p, dt*P+col]
    Wq = nc.dram_tensor("Wq", [DC, P, DC, P], BF16, kind="ExternalInput")
    Wk = nc.dram_tensor("Wk", [DC, P, DC, P], BF16, kind="ExternalInput")
    Wo = nc.dram_tensor("Wo", [DC, P, DC, P], BF16, kind="ExternalInput")
    # Wv pre-tiled as rhs: Wv_t[db][p][c][col] = Wv[c*P+p, db*SB+col]
    Wv = nc.dram_tensor("Wv", [2, P, DC, SB], BF16, kind="ExternalInput")
    bq = nc.dram_tensor("bq", [P, DC], F32, kind="ExternalInput")
    bk = nc.dram_tensor("bk", [P, DC], F32, kind="ExternalInput")
    bv = nc.dram_tensor("bv", [1, D], F32, kind="ExternalInput")
    bo = nc.dram_tensor("bo", [P, DC], F32, kind="ExternalInput")
    outT = nc.dram_tensor("outT", [D, SC], F32, kind="ExternalOutput")

    # local K/V halves + pair-gathered tensors, split by 512-block for
    # finer collective/compute pipelining
    kT_loc = [nc.dram_tensor(f"kT_loc{i}", [D // 2, S // 2], BF16) for i in range(2)]
    v_loc = [nc.dram_tensor(f"v_loc{i}", [SB, D], BF16) for i in range(2)]
    kT_all = [nc.dram_tensor(f"kT_all{i}", [2, D // 2, S // 2], BF16) for i in range(2)]
    v_all = [nc.dram_tensor(f"v_all{i}", [2, SB, D], BF16) for i in range(2)]

    xT_r = xT.rearrange("(c p) s -> p c s", p=P)
    yT_r = yT.rearrange("(c p) t -> p c t", p=P)
    adjT_r = adjT.rearrange("(g p) s -> p g s", p=P)           # g in 0..15
    Wq_r = Wq.rearrange("t p c f -> p t c f")
    Wk_r = Wk.rearrange("t p c f -> p t c f")
    Wo_r = Wo.rearrange("t p c f -> p t c f")
    Wv_r = Wv.rearrange("v p c f -> p v c f")
    kT_all_r = [t.rearrange("r (c p) t -> r p c t", p=P) for t in kT_all]  # c in 0..3
    v_all_r = [t.rearrange("r (j p) d -> r p j d", p=P) for t in v_all]

    with tile.TileContext(nc) as tc:
        with (
            nc.allow_low_precision(reason="bf16 matmuls, fp32 accumulate"),
            tc.tile_pool(name="res", bufs=1) as res,
        ):
            # ---- resident tiles --------------------------------------
            QT_sb = res.tile([P, DC, SC], BF16, name="QT_sb")
            num_sb = res.tile([P, DC, SC], F32, name="num_sb")
            scaled = res.tile([P, NSB, DC, SB], BF16, name="scaled")
            wo_sb = res.tile([P, DC, DC, P], BF16, name="wo_sb")
            recip_sb = res.tile([1, NSB, SB], F32, name="recip_sb")
            denacc = res.tile([P, NSB, SB], F32, name="denacc")
            dsum = res.tile([P, NSB, SB], F32, name="dsum")
            rb = res.tile([P, NSB, SB], F32, name="rb")
            bv_bc = res.tile([P, D], F32, name="bv_bc")
            bq_sb = res.tile([P, DC], F32, name="bq_sb")
            bk_sb = res.tile([P, DC], F32, name="bk_sb")
            bo_sb = res.tile([P, DC], F32, name="bo_sb")
            bv_sb = res.tile([1, D], F32, name="bv_sb")
            nc.sync.dma_start(out=bk_sb[:], in_=bk[:])
            nc.sync.dma_start(out=bv_sb[:], in_=bv[:])
            nc.sync.dma_start(out=bq_sb[:], in_=bq[:])
            nc.sync.dma_start(out=bo_sb[:], in_=bo[:])
            nc.gpsimd.partition_broadcast(bv_bc[:], bv_sb[0:1, :], channels=P)

            with (
                tc.tile_pool(name="qkv_in", bufs=1) as qkvp,
                tc.tile_pool(name="kv_out", bufs=3) as kvo,
                tc.tile_pool(name="qkv_ps", bufs=3, space="PSUM") as qps,
            ):
                yT_sb = qkvp.tile([P, DC, SC], BF16, name="yT_sb")
                xT_sb = qkvp.tile([P, DC, SC], BF16, name="xT_sb")
                wk_sb = qkvp.tile([P, DC, DC, P], BF16, name="wk_sb")
                wq_sb = qkvp.tile([P, DC, DC, P], BF16, name="wq_sb")
                wv_sb = qkvp.tile([P, 2, DC, SB], BF16, name="wv_sb")

                # ---- all input loads, batched, issued up front -------
                # (ordered so the first K-proj matmuls' deps land first)
                for c2 in range(4):
                    cs = slice(2 * c2, 2 * c2 + 2)
                    nc.sync.dma_start(out=yT_sb[:, cs, :], in_=yT_r[:, cs, :])
                nc.sync.dma_start(out=wk_sb[:, 0:1], in_=Wk_r[:, 0:1])
                nc.sync.dma_start(out=wk_sb[:, 1:4], in_=Wk_r[:, 1:4])
                nc.sync.dma_start(out=wk_sb[:, 4:8], in_=Wk_r[:, 4:8])
                nc.sync.dma_start(out=wv_sb[:], in_=Wv_r[:])
                nc.sync.dma_start(out=xT_sb[:, 0:4, :], in_=xT_r[:, 0:4, :])
                nc.sync.dma_start(out=xT_sb[:, 4:8, :], in_=xT_r[:, 4:8, :])
                nc.sync.dma_start(out=wq_sb[:, 0:4], in_=Wq_r[:, 0:4])
                nc.sync.dma_start(out=wq_sb[:, 4:8], in_=Wq_r[:, 4:8])
                nc.sync.dma_start(out=wo_sb[:], in_=Wo_r[:])

                def emit_k(dh):
                    for dt in range(dh * 4, dh * 4 + 4):
                        kt = kvo.tile([P, NSB, SB], BF16, name="kt")
                        for tb in range(NSB):
                            ps = qps.tile([P, SB], F32, name="k_ps", tag="qkvps")
                            for c in range(DC):
                                _mm(
                                    nc, ps[:],
                                    wk_sb[:, dt, c, :],
                                    yT_sb[:, c, tb * SB : (tb + 1) * SB],
                                    c == 0, c == DC - 1,
                                )
                            nc.scalar.activation(
                                out=kt[:, tb, :], in_=ps[:], func=ID,
                                bias=bk_sb[:, dt : dt + 1],
                            )
                        # store rides the scalar queue, right after its ACTs
                        nc.scalar.dma_start(
                            out=kT_loc[dh][(dt - dh * 4) * P : (dt - dh * 4 + 1) * P, :],
                            in_=kt[:],
                        )
                    nc.gpsimd.collective_compute(
                        "AllGather", mybir.AluOpType.bypass,
                        replica_groups=GROUPS,
                        ins=[kT_loc[dh][:]], outs=[kT_all[dh][:]],
                    )

                def emit_v(tb):
                    for tl in range(SB // P):
                        tt = tb * (SB // P) + tl
                        vt = kvo.tile([P, 2, SB], BF16, name="vt")
                        for db in range(2):
                            ps = qps.tile([P, SB], F32, name="v_ps", tag="qkvps")
                            for c in range(DC):
                                _mm(
                                    nc, ps[:],
                                    yT_sb[:, c, tt * P : (tt + 1) * P],
                                    wv_sb[:, db, c, :],
                                    c == 0, c == DC - 1,
                                )
                            nc.vector.tensor_add(
                                vt[:, db, :], ps[:], bv_bc[:, db * SB : (db + 1) * SB]
                            )
                        nc.sync.dma_start(
                            out=v_loc[tb][tl * P : (tl + 1) * P, :],
                            in_=vt[:],
                        )
                    nc.gpsimd.collective_compute(
                        "AllGather", mybir.AluOpType.bypass,
                        replica_groups=GROUPS,
                        ins=[v_loc[tb][:]], outs=[v_all[tb][:]],
                    )

                emit_k(0)
                emit_k(1)
                emit_v(0)
                emit_v(1)

                # ---- phase Q: QT = Wq^T x^T + bq ---------------------
                for dt in range(DC):
                    for sb in range(NSB):
                        ps = qps.tile([P, SB], F32, name="q_ps", tag="qkvps")
                        for c in range(DC):
                            _mm(
                                nc, ps[:],
                                wq_sb[:, dt, c, :],
                                xT_sb[:, c, sb * SB : (sb + 1) * SB],
                                c == 0, c == DC - 1,
                            )
                        nc.scalar.activation(
                            out=QT_sb[:, dt, sb * SB : (sb + 1) * SB],
                            in_=ps[:], func=ID, bias=bq_sb[:, dt : dt + 1],
                        )

            # ---- phase A: attention, t-panel outer -------------------
            # order (r, lb): consume v_all[0] panels first so the second
            # V AllGather has Q-proj + 2 panels of slack to land
            PANELS = [(0, 0), (1, 0), (0, 1), (1, 1)]
            with (
                tc.tile_pool(name="kp_pool", bufs=2) as kpp,
                tc.tile_pool(name="vp_pool", bufs=2) as vpp,
                tc.tile_pool(name="exp_pool", bufs=2) as expp,
                tc.tile_pool(name="adj_pool", bufs=2) as adjp,
                tc.tile_pool(name="tmp_pool", bufs=2) as tmpp,
                tc.tile_pool(name="aps", bufs=3, space="PSUM") as aps,
                tc.tile_pool(name="nps", bufs=5, space="PSUM") as npsp,
            ):
                for pi, (r, lb) in enumerate(PANELS):
                    kp = kpp.tile([P, DC, TP], BF16, name="kp")
                    nc.sync.dma_start(
                        out=kp[:, 0:4, :],
                        in_=kT_all_r[0][r, :, :, lb * TP : (lb + 1) * TP],
                    )
                    nc.sync.dma_start(
                        out=kp[:, 4:8, :],
                        in_=kT_all_r[1][r, :, :, lb * TP : (lb + 1) * TP],
                    )
                    vp = vpp.tile([P, TTP, D], BF16, name="vp")
                    nc.sync.dma_start(out=vp[:], in_=v_all_r[lb][r, :, :, :])
                    g0 = r * 8 + lb * 4
                    for sb in range(NSB):
                        ssl = slice(sb * SB, (sb + 1) * SB)
                        at = adjp.tile([P, TTP, SB], BF16, name="at")
                        nc.sync.dma_start(
                            out=at[:], in_=adjT_r[:, g0 : g0 + 4, ssl]
                        )
                        ex = expp.tile([P, TTP, SB], BF16, name="ex")
                        for tt in range(TTP):
                            att = aps.tile([P, SB], F32, name="att")
                            for c in range(DC):
                                _mm(
                                    nc, att[:],
                                    kp[:, c, tt * P : (tt + 1) * P],
                                    QT_sb[:, c, ssl],
                                    c == 0, c == DC - 1,
                                )
                            tm = tmpp.tile([P, SB], F32, name="tm")
                            nc.vector.tensor_add(tm[:], att[:], at[:, tt, :])
                            nc.scalar.activation(
                                out=ex[:, tt, :], in_=tm[:], func=EXP
                            )
                            if pi == 0 and tt == 0:
                                nc.vector.tensor_copy(denacc[:, sb, :], ex[:, tt, :])
                            else:
                                nc.vector.tensor_add(
                                    denacc[:, sb, :], denacc[:, sb, :], ex[:, tt, :]
                                )
                        # numT partial for this panel, d split in halves
                        for dh in range(2):
                            nt = [
                                npsp.tile([P, SB], F32, name="np")
                                for _ in range(DC // 2)
                            ]
                            for tt in range(TTP):
                                for d4 in range(DC // 2):
                                    _mm(
                                        nc, nt[d4][:],
                                        vp[:, tt,
                                           (dh * 4 + d4) * P : (dh * 4 + d4 + 1) * P],
                                        ex[:, tt, :],
                                        tt == 0, tt == TTP - 1,
                                    )
                            for d4 in range(DC // 2):
                                dst = num_sb[:, dh * 4 + d4, ssl]
                                if pi == 0:
                                    nc.vector.tensor_copy(dst, nt[d4][:])
                                else:
                                    nc.vector.tensor_add(dst, dst, nt[d4][:])
                        if pi == NTP - 1:
                            # gpsimd reduction starts as soon as denacc is
                            # final; the rest of the finalize is emitted
                            # after the loop to keep the DVE queue clear
                            nc.gpsimd.partition_all_reduce(
                                dsum[:, sb, :], denacc[:, sb, :],
                                channels=P, reduce_op=bass_isa.ReduceOp.add,
                            )

                # ---- softmax finalize (off the hot path) -------------
                for sb in range(NSB):
                    ssl = slice(sb * SB, (sb + 1) * SB)
                    nc.vector.reciprocal(recip_sb[0:1, sb, :], dsum[0:1, sb, :])
                    nc.gpsimd.partition_broadcast(
                        rb[:, sb, :], recip_sb[0:1, sb, :], channels=P
                    )
                    for c in range(DC):
                        nc.vector.tensor_mul(
                            scaled[:, sb, c, :],
                            num_sb[:, c, ssl],
                            rb[:, sb, :],
                        )

            # ---- phase O: out^T = Wo^T (numT*recip) + bo -------------
            with (
                tc.tile_pool(name="o_out", bufs=3) as oout,
                tc.tile_pool(name="ops", bufs=3, space="PSUM") as ops,
            ):
                for dt in range(DC):
                    ot = oout.tile([P, NSB, SB], F32, name="ot")
                    for sb in range(NSB):
                        po = ops.tile([P, SB], F32, name="po")
                        for c in range(DC):
                            _mm(
                                nc, po[:],
                                wo_sb[:, dt, c, :],
                                scaled[:, sb, c, :],
                                c == 0, c == DC - 1,
                            )
                        nc.scalar.activation(
                            out=ot[:, sb, :], in_=po[:], func=ID,
                            bias=bo_sb[:, dt : dt + 1],
                        )
                    nc.scalar.dma_start(
                        out=outT[dt * P : (dt + 1) * P, :],
                        in_=ot[:],
                    )
    nc.compile()
    return nc


def _get_nc():
    if "nc" not in _CACHE:
        _CACHE["nc"] = build_nc()
    return _CACHE["nc"]


def _tile_lhs(W):
    # [dt][p][c][col] = W[c*P+p, dt*P+col]
    return np.ascontiguousarray(
        W.reshape(DC, P, DC, P).transpose(2, 1, 0, 3).astype(BF16NP)
    )


def kernel(x, y, adj, Wq, bq, Wk, bk, Wv, bv, Wo, bo, _trace=False):
    x = np.asarray(x, dtype=np.float32)
    y = np.asarray(y, dtype=np.float32)
    adj = np.asarray(adj, dtype=np.float32)
    Wq_h = _tile_lhs(np.asarray(Wq, np.float32) * NORM)
    Wk_h = _tile_lhs(np.asarray(Wk, np.float32))
    Wo_h = _tile_lhs(np.asarray(Wo, np.float32))
    # Wv as rhs tiles: [db][p][c][col] = Wv[c*P+p, db*SB+col]
    Wv_h = np.ascontiguousarray(
        np.asarray(Wv, np.float32).reshape(DC, P, 2, SB).transpose(2, 1, 0, 3)
        .astype(BF16NP)
    )
    bq_s = np.asarray(bq, np.float32) * NORM
    bq_h = np.ascontiguousarray(bq_s.reshape(DC, P).T)
    bk_h = np.ascontiguousarray(np.asarray(bk, np.float32).reshape(DC, P).T)
    bo_h = np.ascontiguousarray(np.asarray(bo, np.float32).reshape(DC, P).T)
    bv_h = np.ascontiguousarray(np.asarray(bv, np.float32).reshape(1, D))

    in_maps = []
    for c in range(8):
        b, h = c // 2, c % 2
        ssl = slice(h * SC, (h + 1) * SC)
        in_maps.append(
            {
                "xT": np.ascontiguousarray(x[b, ssl, :].T.astype(BF16NP)),
                "yT": np.ascontiguousarray(y[b, ssl, :].T.astype(BF16NP)),
                "adjT": np.ascontiguousarray(adj[b, ssl, :].T.astype(BF16NP)),
                "Wq": Wq_h, "Wk": Wk_h, "Wv": Wv_h, "Wo": Wo_h,
                "bq": bq_h, "bk": bk_h, "bv": bv_h, "bo": bo_h,
            }
        )

    nc = _get_nc()
    res = run_bass_kernel_spmd(nc, in_maps, list(range(8)), trace=_trace)
    if _trace:
        _CACHE["last_exec_time_ns"] = res.exec_time_ns
        _CACHE["last_trace"] = (
            res.instructions_and_trace[1] if res.instructions_and_trace else None
        )

    out = np.empty((4, S, D), np.float32)
    for c in range(8):
        b, h = c // 2, c % 2
        out[b, h * SC : (h + 1) * SC, :] = res.results[c]["outT"].T
    return out
